# revision 1
# baseline (speedup 1.0000x reference)
"""Multi-head attention kernel for Trainium2 (Bass/Tile), 8-core SPMD.

Problem: B=4, L=S=2048, H=8, E=D=64, fp32.
  scores = einsum('blhe,bshe->bhls', Q, K) * tau[b] + delta[b]
  A = softmax(scores / sqrt(E), axis=-1)
  out = einsum('bhls,bshd->blhd', A, V)

Key observations:
  - softmax(a*x + c) == softmax(a*x): the per-batch delta bias cancels.
  - attn_mask is all-False / unused by the reference.
  - B*H = 32 (b,h) pairs, each an independent L x S attention block.
    Shard 4 pairs per core across 8 cores; no cross-core comms.

Per-core kernel design (per (b,h) pair), all matmul operands fp16
(1 cycle/row PE streaming; fp32/fp32r stream at half rate), fp32 PSUM:
  - Scores are computed TRANSPOSED: ST[s, l] chunks of [128, 512] so that
    the PV matmul can consume exp(ST) directly as the moving operand with
    full K=128 contraction (no P transposes).
  - QK row-packed: two K=64 matmuls run concurrently in PE row groups
    (0,0)/(64,0); host supplies K^T chunk pairs split across partition
    halves and Q^T duplicated on both halves.
  - exp: ScalarE activation Exp reading multi-bank PSUM groups, with the
    per-batch scale (tau[b]/sqrt(E)) folded into the activation scale.
  - PV: lhsT = V' chunk [s=128, 65] where column 64 is all-ones (computes
    softmax denominators for free), rhs = exp chunk [128, 512],
    accumulated over 16 s-chunks into O^T [65, 512] PSUM.
  - Tail: copy O^T to SBUF, reciprocal of the denominator row broadcast
    across partitions on GpSimd, multiply, store O^T (host transposes).
  - Emission is software-pipelined (QK of unit u+1 before PV of unit u):
    PE executes in program order, so PV's wait on exp must not stall the
    next group's ready QK work.

Host side only reshapes/slices/transposes (sharding + layout); all math
is on-device.
"""

import os
import numpy as np

B, L, S, H, E = 4, 2048, 2048, 8, 64
NCORES = 8
NP = (B * H) // NCORES  # pairs per core = 4

LT = 512          # l-tile size (columns of ST chunks / PV moving dim)
NLT = L // LT     # 4
NSC = S // 128    # 16 s-chunks
# s-chunks per exp group; alternating 4/2 so adjacent groups use different
# PSUM pools (pipelining) while fitting the 8-bank PSUM budget:
# stA(4 banks) + stB(2) + O^T(1) + transpose(1) = 8.
GROUPS = (4, 2, 4, 2, 4)

_PROGRAM = None
LAST_RESULTS = None  # test harness reads exec_time_ns / trace path from here


def _build_program():
    import concourse.bass as bass
    import concourse.bacc as bacc
    import concourse.tile as tile
    from concourse import mybir

    f32 = mybir.dt.float32
    f16 = mybir.dt.float16
    EXP = mybir.ActivationFunctionType.Exp

    nc = bacc.Bacc("TRN2", target_bir_lowering=False, debug=False,
                   num_devices=NCORES)
    # qt2: Q^T duplicated on both partition halves [128, L].
    # kt2: K^T s-chunk pairs split across partition halves:
    #   kt2[0:64, j, :] = K^T chunk 2j, kt2[64:128, j, :] = chunk 2j+1.
    qT = nc.dram_tensor("qt", [NP, 2 * E, L], f32, kind="ExternalInput").ap()
    kT = nc.dram_tensor("kt", [NP, 2 * E, NSC // 2, 128], f32,
                        kind="ExternalInput").ap()
    v = nc.dram_tensor("v", [NP, S, E], f32, kind="ExternalInput").ap()
    taus = nc.dram_tensor("taus", [1, NP], f32, kind="ExternalInput").ap()
    o = nc.dram_tensor("o", [NP, E, L], f32, kind="ExternalOutput").ap()

    with tile.TileContext(nc) as tc:
        from contextlib import ExitStack
        with ExitStack() as ctx:
            consts = ctx.enter_context(tc.tile_pool(name="consts", bufs=1))
            kq_pool = ctx.enter_context(tc.tile_pool(name="kq", bufs=2))
            v_pool = ctx.enter_context(tc.tile_pool(name="vp", bufs=2))
            exp_pool = ctx.enter_context(tc.tile_pool(name="expp", bufs=3))
            tail_pool = ctx.enter_context(tc.tile_pool(name="tail", bufs=2))
            stA_pool = ctx.enter_context(
                tc.tile_pool(name="stA", bufs=1, space="PSUM"))
            stB_pool = ctx.enter_context(
                tc.tile_pool(name="stB", bufs=1, space="PSUM"))
            ot_pool = ctx.enter_context(
                tc.tile_pool(name="ot", bufs=2, space="PSUM"))

            # tau[b] per pair, broadcast to all partitions; fold in 1/sqrt(E)
            tau_bc = consts.tile([128, NP], f32)
            nc.gpsimd.dma_start(out=tau_bc, in_=taus.to_broadcast([128, NP]))
            a_all = consts.tile([128, NP], f32)
            nc.scalar.mul(a_all, tau_bc, 1.0 / np.sqrt(float(E)))

            ones_col = consts.tile([128, 1], f32)
            nc.vector.memset(ones_col, 1.0)

            # Software-pipelined emission: PE executes in program order, so
            # QK(u+1) must be EMITTED before PV(u) — otherwise PV(u)'s wait
            # on exp(u) stalls the ready QK(u+1) behind it in the queue.
            units = []
            for p in range(NP):
                for t in range(NLT):
                    c0 = 0
                    for G in GROUPS:
                        units.append((p, t, G, c0))
                        c0 += G

            pair_tiles = {}

            def emit_loads(p):
                # fp16 operands (1 cyc/row PE stream + fast weight load);
                # gpsimd DMA casts f32 -> f16 in flight
                kt_sb = kq_pool.tile([128, NSC // 2, 128], f16, tag="kt")
                nc.gpsimd.dma_start(out=kt_sb, in_=kT[p])
                qt_sb = kq_pool.tile([128, L], f16, tag="qt")
                nc.gpsimd.dma_start(out=qt_sb, in_=qT[p])
                # V chunks [s=128, 65], col 64 = 1.0 (denominator trick)
                vp_sb = v_pool.tile([128, NSC, E + 1], f16, tag="vp")
                nc.vector.tensor_copy(vp_sb[:, :, E:E + 1],
                                      ones_col.to_broadcast([128, NSC, 1]))
                nc.gpsimd.dma_start(
                    out=vp_sb[:, :, 0:E],
                    in_=v[p].rearrange("(n q) e -> q n e", q=128))
                pair_tiles[p] = (kt_sb, qt_sb, vp_sb)

            def emit_qk(u):
                p, t, G, c0 = u
                kt_sb, qt_sb, _ = pair_tiles[p]
                pool = stA_pool if G == 4 else stB_pool
                st_ps = pool.tile([128, G * LT], f32)
                for k2 in range(G // 2):
                    j = (c0 // 2) + k2  # packed chunk-pair index
                    nc.tensor.matmul(
                        st_ps[:, (2 * k2) * LT:(2 * k2 + 1) * LT],
                        lhsT=kt_sb[0:64, j, :],
                        rhs=qt_sb[0:64, t * LT:(t + 1) * LT],
                        start=True, stop=True, tile_position=(0, 0))
                    nc.tensor.matmul(
                        st_ps[:, (2 * k2 + 1) * LT:(2 * k2 + 2) * LT],
                        lhsT=kt_sb[64:128, j, :],
                        rhs=qt_sb[64:128, t * LT:(t + 1) * LT],
                        start=True, stop=True, tile_position=(64, 0))
                return st_ps

            cur_ot = [None]

            def emit_pv(u, ex):
                p, t, G, c0 = u
                vp_sb = pair_tiles[p][2]
                if c0 == 0:
                    cur_ot[0] = ot_pool.tile([E + 1, LT], f32, name="ot_ps",
                                             tag="ot_ps")
                for k in range(G):
                    c = c0 + k
                    nc.tensor.matmul(
                        cur_ot[0],
                        lhsT=vp_sb[:, c, :],
                        rhs=ex[:, k * LT:(k + 1) * LT],
                        start=(c == 0), stop=(c == NSC - 1))

            def emit_tail(u):
                p, t, G, c0 = u
                # Normalize O^T by its denominator row, store O^T (host
                # transposes back). recip row partition-broadcast on GpSimd.
                ot_sb = tail_pool.tile([E + 1, LT], f32, tag="otsb")
                nc.vector.tensor_copy(ot_sb, cur_ot[0])
                rs_row = tail_pool.tile([1, LT], f32, tag="rsrow")
                nc.vector.reciprocal(rs_row, ot_sb[E:E + 1, :])
                rb = tail_pool.tile([64, LT], f32, tag="rb")
                nc.gpsimd.partition_broadcast(rb, rs_row, channels=64)
                on = tail_pool.tile([64, LT], f32, tag="on")
                nc.vector.tensor_mul(on, ot_sb[0:E, :], rb)
                nc.sync.dma_start(out=o[p, :, t * LT:(t + 1) * LT], in_=on)

            emit_loads(0)
            st_cur = emit_qk(units[0])
            for i, u in enumerate(units):
                p, t, G, c0 = u
                ex = exp_pool.tile([128, 4 * LT], f16, tag="ex")
                nc.scalar.activation(ex[:, 0:G * LT], st_cur, EXP,
                                     scale=a_all[:, p:p + 1])
                if i + 1 < len(units):
                    nxt = units[i + 1]
                    if nxt[0] != p:
                        emit_loads(nxt[0])
                    st_cur = emit_qk(nxt)
                emit_pv(u, ex)
                if c0 + G == NSC:  # last group of this l-tile
                    emit_tail(u)
    nc.compile()
    return nc


def _get_program():
    global _PROGRAM
    if _PROGRAM is None:
        _PROGRAM = _build_program()
    return _PROGRAM


def kernel(queries, keys, values, attn_mask=None, tau=None, delta=None):
    from concourse.bass_utils import run_bass_kernel_spmd

    queries = np.ascontiguousarray(np.asarray(queries, dtype=np.float32))
    keys = np.ascontiguousarray(np.asarray(keys, dtype=np.float32))
    values = np.ascontiguousarray(np.asarray(values, dtype=np.float32))
    tau_flat = np.asarray(tau, dtype=np.float32).reshape(B)

    # pair = b*H + h; per-pair transposed layouts (host does layout only)
    qT_base = queries.transpose(0, 2, 3, 1).reshape(B * H, E, L)
    qT_all = np.ascontiguousarray(
        np.concatenate([qT_base, qT_base], axis=1))  # [32, 128, L] duplicated
    kT_base = keys.transpose(0, 2, 3, 1).reshape(B * H, E, S)
    kc = kT_base.reshape(B * H, E, S // 128, 128)
    kT_all = np.ascontiguousarray(
        np.concatenate([kc[:, :, 0::2, :], kc[:, :, 1::2, :]], axis=1))
    # kT_all: [32, 128, 8, 128]; rows 0:64 = even chunks, 64:128 = odd
    v_all = np.ascontiguousarray(
        values.transpose(0, 2, 1, 3).reshape(B * H, S, E))

    nc = _get_program()
    in_maps = []
    for c in range(NCORES):
        lo = c * NP
        tau_pairs = np.ascontiguousarray(
            tau_flat[[(lo + i) // H for i in range(NP)]].reshape(1, NP))
        in_maps.append({
            "qt": qT_all[lo:lo + NP],
            "kt": kT_all[lo:lo + NP],
            "v": v_all[lo:lo + NP],
            "taus": tau_pairs,
        })

    kwargs = {}
    if os.environ.get("ATTN_TRACE"):
        kwargs["trace"] = True
        tmpdir = os.environ.get("ATTN_TRACE_DIR")
        if tmpdir:
            os.makedirs(tmpdir, exist_ok=True)
            kwargs["tmpdir"] = tmpdir

    res = run_bass_kernel_spmd(nc, in_maps, list(range(NCORES)), **kwargs)
    global LAST_RESULTS
    LAST_RESULTS = res

    o_all = np.concatenate([r["o"] for r in res.results], axis=0)  # [32, E, L]
    out = o_all.reshape(B, H, E, L).transpose(0, 3, 1, 2)  # [B, L, H, E]
    return np.ascontiguousarray(out)



# revision 3
# speedup vs baseline: 1.1427x; 1.1427x over previous
"""Multi-head attention kernel for Trainium2 (Bass/Tile), 8-core SPMD.

Problem: B=4, L=S=2048, H=8, E=D=64, fp32.
  scores = einsum('blhe,bshe->bhls', Q, K) * tau[b] + delta[b]
  A = softmax(scores / sqrt(E), axis=-1)
  out = einsum('bhls,bshd->blhd', A, V)

Key observations:
  - softmax(a*x + c) == softmax(a*x): the per-batch delta bias cancels.
  - attn_mask is all-False / unused by the reference.
  - B*H = 32 (b,h) pairs, each an independent L x S attention block.
    Shard 4 pairs per core across 8 cores; no cross-core comms.

Per-core kernel design (per (b,h) pair), all matmul operands fp16
(1 cycle/row PE streaming), fp32 PSUM:
  - Scores are computed TRANSPOSED: ST[s, l] chunks of [128, 512] so that
    the PV matmul can consume exp(ST) directly as the moving operand with
    full K=128 contraction (no P transposes).
  - QK row-packed: two K=64 matmuls run concurrently in PE row groups
    (0,0)/(64,0); host supplies K^T chunk pairs split across partition
    halves and Q^T duplicated on both halves.
  - exp is SPLIT between ScalarE (exact LUT exp, 10 of 16 s-chunks per
    l-tile) and VectorE (6 of 16 chunks) to break the ScalarE throughput
    wall (ScalarE alone = 1 elem/cyc/lane = 109us/core). The DVE side
    uses a bias-corrected Schraudolph bit-trick: a single tensor_scalar
    (mult+add, fp32 in, int16 round-convert out) produces the fp16 BIT
    PATTERN of 2^(y+c); c = -0.0561 zeroes the mean log-bias so the
    sawtooth error (~2% rms on 6/16 of weights -> ~1% final L2) does not
    shift attention mass between exact and approx chunks.
  - Score PSUM: two 3-bank pools (A,B) alternate across six groups per
    l-tile (sizes 3,3,2,2,3,3); consumers (S,S,S,S,D,D) are placed so
    each pool's reuse chain interleaves ScalarE/DVE work and stays off
    the critical path. PSUM: stA(3) + stB(3) + ot(2x1) = 8 banks.
  - PV: lhsT = V' chunk [s=128, 65] where column 64 is all-ones (computes
    softmax denominators for free), rhs = exp chunk [128, 512],
    accumulated over 16 s-chunks into O^T [65, 512] PSUM.
  - Tail: copy the denominator row to SBUF, reciprocal_approx_fast (one
    custom-DVE op, ~5x faster than iterative reciprocal), GpSimd
    partition-broadcast, multiply straight out of PSUM, store O^T (host
    transposes).
  - Emission is software-pipelined (QK of unit u+1 before PV of unit u):
    PE executes in program order, so PV's wait on exp must not stall the
    next group's ready QK work.
  - Ramp: input loads are chunked (kt 2, qt 4, vp 2 pieces) so the first
    QK starts after ~1/4 of pair-0 bytes; a dummy exp warms the ACT
    table during the load; pair p+1 loads prefetch at pair p's last
    l-tile.

Host side only reshapes/slices/transposes (sharding + layout); all math
is on-device.
"""

import os
import numpy as np

B, L, S, H, E = 4, 2048, 2048, 8, 64
NCORES = 8
NP = (B * H) // NCORES  # pairs per core = 4

LT = 512          # l-tile size (columns of ST chunks / PV moving dim)
NLT = L // LT     # 4
NSC = S // 128    # 16 s-chunks

# Six exp groups per l-tile: (chunks, psum pool, consumer engine).
# Pools A/B are 3 PSUM banks each; consumers: 'S' = ScalarE exact exp,
# 'D' = DVE Schraudolph.
GROUPS = (
    ([0, 1, 2], "A", "S"),
    ([3, 4, 5], "B", "S"),
    ([6, 7], "A", "S"),
    ([8, 9], "B", "S"),
    ([10, 11, 12], "A", "D"),
    ([13, 14, 15], "B", "D"),
)
# kt slot layout: slot j holds (top chunk, bottom chunk) in partition
# halves 0:64 / 64:128. Pairs grouped so each exp group's chunks pack.
KT_TOP = [0, 2, 3, 6, 8, 10, 12, 13]
KT_BOT = [1, 5, 4, 7, 9, 11, 15, 14]
CHUNK_SLOT = {}
for _j, (_t, _b) in enumerate(zip(KT_TOP, KT_BOT)):
    CHUNK_SLOT[_t] = (_j, 0)
    CHUNK_SLOT[_b] = (_j, 1)

LOG2E = 1.4426950408889634
SCH_C = -0.0561  # bias-free Schraudolph constant (round-to-nearest)
SCH_BIAS = (15.0 + SCH_C) * 1024.0

_PROGRAM = None
LAST_RESULTS = None  # test harness reads exec_time_ns / trace path from here


def _build_program():
    import concourse.bass as bass
    import concourse.bacc as bacc
    import concourse.tile as tile
    from concourse import mybir

    f32 = mybir.dt.float32
    f16 = mybir.dt.float16
    i16 = mybir.dt.int16
    EXP = mybir.ActivationFunctionType.Exp
    MUL = mybir.AluOpType.mult
    ADD = mybir.AluOpType.add

    nc = bacc.Bacc("TRN2", target_bir_lowering=False, debug=False,
                   num_devices=NCORES)
    # qt2: Q^T duplicated on both partition halves [128, L].
    # kt: K^T s-chunks packed per KT_TOP/KT_BOT slots.
    qT = nc.dram_tensor("qt", [NP, 2 * E, L], f32, kind="ExternalInput").ap()
    kT = nc.dram_tensor("kt", [NP, 2 * E, NSC // 2, 128], f32,
                        kind="ExternalInput").ap()
    v = nc.dram_tensor("v", [NP, S, E], f32, kind="ExternalInput").ap()
    taus = nc.dram_tensor("taus", [1, NP], f32, kind="ExternalInput").ap()
    o = nc.dram_tensor("o", [NP, E, L], f32, kind="ExternalOutput").ap()

    with tile.TileContext(nc) as tc:
        from contextlib import ExitStack
        with ExitStack() as ctx:
            consts = ctx.enter_context(tc.tile_pool(name="consts", bufs=1))
            kq_pool = ctx.enter_context(tc.tile_pool(name="kq", bufs=2))
            v_pool = ctx.enter_context(tc.tile_pool(name="vp", bufs=2))
            exp_pool = ctx.enter_context(tc.tile_pool(name="expp", bufs=3))
            tail_pool = ctx.enter_context(tc.tile_pool(name="tail", bufs=2))
            stA_pool = ctx.enter_context(
                tc.tile_pool(name="stA", bufs=1, space="PSUM"))
            stB_pool = ctx.enter_context(
                tc.tile_pool(name="stB", bufs=1, space="PSUM"))
            ot_pool = ctx.enter_context(
                tc.tile_pool(name="ot", bufs=2, space="PSUM"))

            # Warm the exp ACT table set while the first loads run.
            warm = consts.tile([128, 1], f32)
            nc.vector.memset(warm, 0.0)
            warm2 = consts.tile([128, 1], f32)
            nc.scalar.activation(warm2, warm, EXP)

            # Per-pair constants from tau[b]: ACT scale tau/sqrt(E) and the
            # Schraudolph multiplier tau/sqrt(E)*log2(e)*1024.
            tau_bc = consts.tile([128, NP], f32)
            nc.gpsimd.dma_start(out=tau_bc, in_=taus.to_broadcast([128, NP]))
            a_act = consts.tile([128, NP], f32)
            nc.scalar.mul(a_act, tau_bc, 1.0 / np.sqrt(float(E)))
            a_ts = consts.tile([128, NP], f32)
            nc.scalar.mul(a_ts, tau_bc, LOG2E * 1024.0 / np.sqrt(float(E)))

            ones_col = consts.tile([128, 1], f32)
            nc.vector.memset(ones_col, 1.0)

            units = []
            for p in range(NP):
                for t in range(NLT):
                    for gi in range(len(GROUPS)):
                        units.append((p, t, gi))

            pair_tiles = {}

            def emit_loads(p):
                # fp16 operands; gpsimd DMA casts f32 -> f16 in flight.
                # Chunked so the first QK only waits on the first pieces.
                kt_sb = kq_pool.tile([128, NSC // 2, 128], f16, tag="kt")
                nc.gpsimd.dma_start(out=kt_sb[:, 0:4, :], in_=kT[p][:, 0:4, :])
                nc.gpsimd.dma_start(out=kt_sb[:, 4:8, :], in_=kT[p][:, 4:8, :])
                qt_sb = kq_pool.tile([128, L], f16, tag="qt")
                for t in range(NLT):
                    nc.gpsimd.dma_start(out=qt_sb[:, t * LT:(t + 1) * LT],
                                        in_=qT[p][:, t * LT:(t + 1) * LT])
                # V chunks [s=128, 65], col 64 = 1.0 (denominator trick)
                vp_sb = v_pool.tile([128, NSC, E + 1], f16, tag="vp")
                nc.vector.tensor_copy(vp_sb[:, :, E:E + 1],
                                      ones_col.to_broadcast([128, NSC, 1]))
                vre = v[p].rearrange("(n q) e -> q n e", q=128)
                nc.gpsimd.dma_start(out=vp_sb[:, 0:8, 0:E], in_=vre[:, 0:8, :])
                nc.gpsimd.dma_start(out=vp_sb[:, 8:16, 0:E],
                                    in_=vre[:, 8:16, :])
                pair_tiles[p] = (kt_sb, qt_sb, vp_sb)

            def emit_qk(u):
                p, t, gi = u
                chunks, pool_name, _ = GROUPS[gi]
                kt_sb, qt_sb, _ = pair_tiles[p]
                pool = stA_pool if pool_name == "A" else stB_pool
                tag = "stA" if pool_name == "A" else "stB"
                st_ps = pool.tile([128, len(chunks) * LT], f32, tag=tag)
                rhs_t = qt_sb[0:64, t * LT:(t + 1) * LT]
                rhs_b = qt_sb[64:128, t * LT:(t + 1) * LT]
                for pos, c in enumerate(chunks):
                    slot, half = CHUNK_SLOT[c]
                    dst = st_ps[:, pos * LT:(pos + 1) * LT]
                    if half == 0:
                        nc.tensor.matmul(dst, lhsT=kt_sb[0:64, slot, :],
                                         rhs=rhs_t, start=True, stop=True,
                                         tile_position=(0, 0))
                    else:
                        nc.tensor.matmul(dst, lhsT=kt_sb[64:128, slot, :],
                                         rhs=rhs_b, start=True, stop=True,
                                         tile_position=(64, 0))
                return st_ps

            cur_ot = [None]

            def emit_exp(u, st_ps):
                p, t, gi = u
                chunks, _, eng = GROUPS[gi]
                n = len(chunks) * LT
                ex = exp_pool.tile([128, 3 * LT], f16, tag="ex")
                if eng == "S":
                    nc.scalar.activation(ex[:, 0:n], st_ps, EXP,
                                         scale=a_act[:, p:p + 1])
                else:
                    # Schraudolph: fp16 bits of 2^(score*a*log2e + 15+c)
                    # via fp32 affine + round-to-nearest int16 convert.
                    ex_i = ex.bitcast(i16)
                    nc.vector.tensor_scalar(
                        ex_i[:, 0:n], st_ps, a_ts[:, p:p + 1], SCH_BIAS,
                        MUL, ADD)
                return ex

            def emit_pv(u, ex):
                p, t, gi = u
                chunks, _, _ = GROUPS[gi]
                vp_sb = pair_tiles[p][2]
                if gi == 0:
                    cur_ot[0] = ot_pool.tile([E + 1, LT], f32, name="ot_ps",
                                             tag="ot_ps")
                for pos, c in enumerate(chunks):
                    nc.tensor.matmul(
                        cur_ot[0],
                        lhsT=vp_sb[:, c, :],
                        rhs=ex[:, pos * LT:(pos + 1) * LT],
                        start=(c == 0), stop=(c == NSC - 1))

            def emit_tail(u):
                p, t, gi = u
                # Normalize O^T by its denominator row, store O^T (host
                # transposes back). recip row partition-broadcast on GpSimd.
                den_row = tail_pool.tile([1, LT], f32, tag="drow")
                nc.vector.tensor_copy(den_row, cur_ot[0][E:E + 1, :])
                rs_row = tail_pool.tile([1, LT], f32, tag="rsrow")
                nc.vector.reciprocal_approx_fast(rs_row, den_row)
                rb = tail_pool.tile([64, LT], f32, tag="rb")
                nc.gpsimd.partition_broadcast(rb, rs_row, channels=64)
                on = tail_pool.tile([64, LT], f32, tag="on")
                nc.vector.tensor_mul(on, cur_ot[0][0:E, :], rb)
                nc.sync.dma_start(out=o[p, :, t * LT:(t + 1) * LT], in_=on)

            emit_loads(0)
            st_cur = emit_qk(units[0])
            for i, u in enumerate(units):
                p, t, gi = u
                ex = emit_exp(u, st_cur)
                if i + 1 < len(units):
                    nxt = units[i + 1]
                    # Prefetch next pair's inputs at this pair's last l-tile.
                    if (t, gi) == (NLT - 1, 0) and p + 1 < NP:
                        emit_loads(p + 1)
                    st_cur = emit_qk(nxt)
                emit_pv(u, ex)
                if gi == len(GROUPS) - 1:
                    emit_tail(u)
    nc.compile()
    return nc


def _get_program():
    global _PROGRAM
    if _PROGRAM is None:
        _PROGRAM = _build_program()
    return _PROGRAM


def kernel(queries, keys, values, attn_mask=None, tau=None, delta=None):
    from concourse.bass_utils import run_bass_kernel_spmd

    queries = np.ascontiguousarray(np.asarray(queries, dtype=np.float32))
    keys = np.ascontiguousarray(np.asarray(keys, dtype=np.float32))
    values = np.ascontiguousarray(np.asarray(values, dtype=np.float32))
    tau_flat = np.asarray(tau, dtype=np.float32).reshape(B)

    # pair = b*H + h; per-pair transposed layouts (host does layout only)
    qT_base = queries.transpose(0, 2, 3, 1).reshape(B * H, E, L)
    qT_all = np.ascontiguousarray(
        np.concatenate([qT_base, qT_base], axis=1))  # [32, 128, L] duplicated
    kT_base = keys.transpose(0, 2, 3, 1).reshape(B * H, E, S)
    kc = kT_base.reshape(B * H, E, S // 128, 128)
    kT_all = np.ascontiguousarray(
        np.concatenate([kc[:, :, KT_TOP, :], kc[:, :, KT_BOT, :]], axis=1))
    # kT_all: [32, 128, 8, 128]; rows 0:64 = KT_TOP chunks, 64:128 = KT_BOT
    v_all = np.ascontiguousarray(
        values.transpose(0, 2, 1, 3).reshape(B * H, S, E))

    nc = _get_program()
    in_maps = []
    for c in range(NCORES):
        lo = c * NP
        tau_pairs = np.ascontiguousarray(
            tau_flat[[(lo + i) // H for i in range(NP)]].reshape(1, NP))
        in_maps.append({
            "qt": qT_all[lo:lo + NP],
            "kt": kT_all[lo:lo + NP],
            "v": v_all[lo:lo + NP],
            "taus": tau_pairs,
        })

    kwargs = {}
    if os.environ.get("ATTN_TRACE"):
        kwargs["trace"] = True
        tmpdir = os.environ.get("ATTN_TRACE_DIR")
        if tmpdir:
            os.makedirs(tmpdir, exist_ok=True)
            kwargs["tmpdir"] = tmpdir

    res = run_bass_kernel_spmd(nc, in_maps, list(range(NCORES)), **kwargs)
    global LAST_RESULTS
    LAST_RESULTS = res

    o_all = np.concatenate([r["o"] for r in res.results], axis=0)  # [32, E, L]
    out = o_all.reshape(B, H, E, L).transpose(0, 3, 1, 2)  # [B, L, H, E]
    return np.ascontiguousarray(out)


# revision 4
# speedup vs baseline: 1.2579x; 1.1008x over previous
"""Multi-head attention kernel for Trainium2 (Bass/Tile), 8-core SPMD.

Problem: B=4, L=S=2048, H=8, E=D=64, fp32.
  scores = einsum('blhe,bshe->bhls', Q, K) * tau[b] + delta[b]
  A = softmax(scores / sqrt(E), axis=-1)
  out = einsum('bhls,bshd->blhd', A, V)

Key observations:
  - softmax(a*x + c) == softmax(a*x): the per-batch delta bias cancels.
  - attn_mask is all-False / unused by the reference.
  - B*H = 32 (b,h) pairs, each an independent L x S attention block.
    Shard 4 pairs per core across 8 cores; no cross-core comms.

Per-core kernel design (per (b,h) pair), all matmul operands fp16
(1 cycle/row PE streaming), fp32 PSUM:
  - Scores are computed TRANSPOSED: ST[s, l] chunks of [128, 512] so that
    the PV matmul can consume exp(ST) directly as the moving operand with
    full K=128 contraction (no P transposes).
  - QK row-packed: two K=64 matmuls run concurrently in PE row groups
    (0,0)/(64,0); host supplies K^T chunk pairs split across partition
    halves and Q^T duplicated on both halves.
  - exp is SPLIT between ScalarE (exact LUT exp, 10 of 16 s-chunks per
    l-tile) and VectorE (6 of 16) to break the ScalarE throughput wall
    (ScalarE alone = 1 elem/cyc/lane = 109us/core). The DVE side uses a
    bias-corrected Schraudolph bit-trick: one tensor_scalar (mult+add,
    fp32 in, int16 round-convert out) produces the fp16 BIT PATTERN of
    2^(y+c); c = -0.0561 zeroes the mean log-bias so the sawtooth error
    (~2% rms on 6/16 of the weights -> ~1% final L2) does not shift
    attention mass between exact and approx chunks.
  - Score PSUM: three 2-bank pools rotate over eight 2-chunk groups per
    l-tile (pool = global group index % 3, so reuse distance is always 3
    groups and no consumer sits on the next tile's critical path).
    Consumer pattern per tile: S,S,D,S,S,D,S,D. PSUM budget:
    3x2 (scores) + 2x1 (O^T) = 8 banks.
  - PV: lhsT = V' chunk [s=128, 65] where column 64 is all-ones (computes
    softmax denominators for free), rhs = exp chunk [128, 512],
    accumulated over 16 s-chunks into O^T [65, 512] PSUM.
  - Tail: copy the denominator row to SBUF, reciprocal_approx_fast (one
    custom-DVE op, ~5x faster than iterative reciprocal), GpSimd
    partition-broadcast, multiply straight out of PSUM, store O^T (host
    transposes).
  - Emission is software-pipelined (QK of unit u+1 before PV of unit u):
    PE executes in program order, so PV's wait on exp must not stall the
    next group's ready QK work.
  - Ramp: pair-0 loads are chunked (kt 2, qt 4, vp 2 pieces) so the
    first QK starts after ~1/4 of the bytes; later pairs use single
    DMAs (less GpSimd descriptor-gen) prefetched a full l-tile ahead;
    a dummy exp warms the ACT table during the initial load.

Host side only reshapes/slices/transposes (sharding + layout); all math
is on-device.
"""

import os
import numpy as np

B, L, S, H, E = 4, 2048, 2048, 8, 64
NCORES = 8
NP = (B * H) // NCORES  # pairs per core = 4

LT = 512          # l-tile size (columns of ST chunks / PV moving dim)
NLT = L // LT     # 4
NSC = S // 128    # 16 s-chunks
NG = 8            # exp groups per l-tile (2 chunks each)
# Consumers per tile: 'S' = ScalarE exact exp, 'D' = DVE Schraudolph.
CONSUMERS = "SSDSSDSD"

LOG2E = 1.4426950408889634
SCH_C = -0.0561  # bias-free Schraudolph constant (round-to-nearest)
SCH_BIAS = (15.0 + SCH_C) * 1024.0

_PROGRAM = None
LAST_RESULTS = None  # test harness reads exec_time_ns / trace path from here


def _build_program():
    import concourse.bass as bass
    import concourse.bacc as bacc
    import concourse.tile as tile
    from concourse import mybir

    f32 = mybir.dt.float32
    f16 = mybir.dt.float16
    i16 = mybir.dt.int16
    EXP = mybir.ActivationFunctionType.Exp
    MUL = mybir.AluOpType.mult
    ADD = mybir.AluOpType.add

    nc = bacc.Bacc("TRN2", target_bir_lowering=False, debug=False,
                   num_devices=NCORES)
    # qt: Q^T duplicated on both partition halves [128, L].
    # kt: K^T chunk pairs: slot j = (chunk 2j on rows 0:64, 2j+1 on 64:128).
    qT = nc.dram_tensor("qt", [NP, 2 * E, L], f32, kind="ExternalInput").ap()
    kT = nc.dram_tensor("kt", [NP, 2 * E, NSC // 2, 128], f32,
                        kind="ExternalInput").ap()
    v = nc.dram_tensor("v", [NP, S, E], f32, kind="ExternalInput").ap()
    taus = nc.dram_tensor("taus", [1, NP], f32, kind="ExternalInput").ap()
    o = nc.dram_tensor("o", [NP, E, L], f32, kind="ExternalOutput").ap()

    with tile.TileContext(nc) as tc:
        from contextlib import ExitStack
        with ExitStack() as ctx:
            consts = ctx.enter_context(tc.tile_pool(name="consts", bufs=1))
            kq_pool = ctx.enter_context(tc.tile_pool(name="kq", bufs=2))
            v_pool = ctx.enter_context(tc.tile_pool(name="vp", bufs=2))
            exp_pool = ctx.enter_context(tc.tile_pool(name="expp", bufs=3))
            tail_pool = ctx.enter_context(tc.tile_pool(name="tail", bufs=2))
            st_pools = [
                ctx.enter_context(
                    tc.tile_pool(name=f"st{chr(65 + k)}", bufs=1,
                                 space="PSUM"))
                for k in range(3)
            ]
            ot_pool = ctx.enter_context(
                tc.tile_pool(name="ot", bufs=2, space="PSUM"))

            # Warm the exp ACT table set while the first loads run.
            warm = consts.tile([128, 1], f32)
            nc.vector.memset(warm, 0.0)
            warm2 = consts.tile([128, 1], f32)
            nc.scalar.activation(warm2, warm, EXP)

            # Per-pair constants from tau[b]: ACT scale tau/sqrt(E) and the
            # Schraudolph multiplier tau/sqrt(E)*log2(e)*1024.
            tau_bc = consts.tile([128, NP], f32)
            nc.gpsimd.dma_start(out=tau_bc, in_=taus.to_broadcast([128, NP]))
            a_act = consts.tile([128, NP], f32)
            nc.scalar.mul(a_act, tau_bc, 1.0 / np.sqrt(float(E)))
            a_ts = consts.tile([128, NP], f32)
            nc.scalar.mul(a_ts, tau_bc, LOG2E * 1024.0 / np.sqrt(float(E)))

            ones_col = consts.tile([128, 1], f32)
            nc.vector.memset(ones_col, 1.0)

            units = []
            for p in range(NP):
                for t in range(NLT):
                    for gi in range(NG):
                        units.append((p, t, gi))

            pair_tiles = {}

            def emit_loads(p, chunked):
                # fp16 operands; gpsimd DMA casts f32 -> f16 in flight.
                # Pair 0 is chunked so the first QK starts early; later
                # pairs prefetch with single descriptors.
                kt_sb = kq_pool.tile([128, NSC // 2, 128], f16, tag="kt")
                qt_sb = kq_pool.tile([128, L], f16, tag="qt")
                vp_sb = v_pool.tile([128, NSC, E + 1], f16, tag="vp")
                nc.vector.tensor_copy(vp_sb[:, :, E:E + 1],
                                      ones_col.to_broadcast([128, NSC, 1]))
                vre = v[p].rearrange("(n q) e -> q n e", q=128)
                if chunked:
                    nc.gpsimd.dma_start(out=kt_sb[:, 0:4, :],
                                        in_=kT[p][:, 0:4, :])
                    nc.gpsimd.dma_start(out=kt_sb[:, 4:8, :],
                                        in_=kT[p][:, 4:8, :])
                    for t in range(NLT):
                        nc.gpsimd.dma_start(
                            out=qt_sb[:, t * LT:(t + 1) * LT],
                            in_=qT[p][:, t * LT:(t + 1) * LT])
                    nc.gpsimd.dma_start(out=vp_sb[:, 0:8, 0:E],
                                        in_=vre[:, 0:8, :])
                    nc.gpsimd.dma_start(out=vp_sb[:, 8:16, 0:E],
                                        in_=vre[:, 8:16, :])
                else:
                    nc.gpsimd.dma_start(out=kt_sb, in_=kT[p])
                    nc.gpsimd.dma_start(out=qt_sb, in_=qT[p])
                    nc.gpsimd.dma_start(out=vp_sb[:, :, 0:E], in_=vre)
                pair_tiles[p] = (kt_sb, qt_sb, vp_sb)

            gctr = [0]  # global group counter -> score pool rotation

            def emit_qk(u):
                p, t, gi = u
                kt_sb, qt_sb, _ = pair_tiles[p]
                pool = st_pools[gctr[0] % 3]
                gctr[0] += 1
                st_ps = pool.tile([128, 2 * LT], f32, tag="st")
                nc.tensor.matmul(
                    st_ps[:, 0:LT],
                    lhsT=kt_sb[0:64, gi, :],
                    rhs=qt_sb[0:64, t * LT:(t + 1) * LT],
                    start=True, stop=True, tile_position=(0, 0))
                nc.tensor.matmul(
                    st_ps[:, LT:2 * LT],
                    lhsT=kt_sb[64:128, gi, :],
                    rhs=qt_sb[64:128, t * LT:(t + 1) * LT],
                    start=True, stop=True, tile_position=(64, 0))
                return st_ps

            cur_ot = [None]

            def emit_exp(u, st_ps):
                p, t, gi = u
                ex = exp_pool.tile([128, 2 * LT], f16, tag="ex")
                if CONSUMERS[gi] == "S":
                    nc.scalar.activation(ex, st_ps, EXP,
                                         scale=a_act[:, p:p + 1])
                else:
                    # Schraudolph: fp16 bits of 2^(score*a*log2e + 15+c)
                    # via fp32 affine + round-to-nearest int16 convert.
                    nc.vector.tensor_scalar(
                        ex.bitcast(i16), st_ps, a_ts[:, p:p + 1], SCH_BIAS,
                        MUL, ADD)
                return ex

            def emit_pv(u, ex):
                p, t, gi = u
                vp_sb = pair_tiles[p][2]
                if gi == 0:
                    cur_ot[0] = ot_pool.tile([E + 1, LT], f32, name="ot_ps",
                                             tag="ot_ps")
                for k in range(2):
                    c = 2 * gi + k
                    nc.tensor.matmul(
                        cur_ot[0],
                        lhsT=vp_sb[:, c, :],
                        rhs=ex[:, k * LT:(k + 1) * LT],
                        start=(c == 0), stop=(c == NSC - 1))

            def emit_tail(u):
                p, t, gi = u
                # Normalize O^T by its denominator row, store O^T (host
                # transposes back). recip row partition-broadcast on GpSimd.
                den_row = tail_pool.tile([1, LT], f32, tag="drow")
                nc.vector.tensor_copy(den_row, cur_ot[0][E:E + 1, :])
                rs_row = tail_pool.tile([1, LT], f32, tag="rsrow")
                nc.vector.reciprocal_approx_fast(rs_row, den_row)
                rb = tail_pool.tile([64, LT], f32, tag="rb")
                nc.gpsimd.partition_broadcast(rb, rs_row, channels=64)
                on = tail_pool.tile([64, LT], f32, tag="on")
                nc.vector.tensor_mul(on, cur_ot[0][0:E, :], rb)
                nc.sync.dma_start(out=o[p, :, t * LT:(t + 1) * LT], in_=on)

            emit_loads(0, chunked=True)
            st_cur = emit_qk(units[0])
            for i, u in enumerate(units):
                p, t, gi = u
                ex = emit_exp(u, st_cur)
                if i + 1 < len(units):
                    nxt = units[i + 1]
                    # Prefetch next pair's inputs at this pair's last l-tile.
                    if (t, gi) == (NLT - 1, 0) and p + 1 < NP:
                        emit_loads(p + 1, chunked=False)
                    st_cur = emit_qk(nxt)
                emit_pv(u, ex)
                if gi == NG - 1:
                    emit_tail(u)
    nc.compile()
    return nc


def _get_program():
    global _PROGRAM
    if _PROGRAM is None:
        _PROGRAM = _build_program()
    return _PROGRAM


def kernel(queries, keys, values, attn_mask=None, tau=None, delta=None):
    from concourse.bass_utils import run_bass_kernel_spmd

    queries = np.ascontiguousarray(np.asarray(queries, dtype=np.float32))
    keys = np.ascontiguousarray(np.asarray(keys, dtype=np.float32))
    values = np.ascontiguousarray(np.asarray(values, dtype=np.float32))
    tau_flat = np.asarray(tau, dtype=np.float32).reshape(B)

    # pair = b*H + h; per-pair transposed layouts (host does layout only)
    qT_base = queries.transpose(0, 2, 3, 1).reshape(B * H, E, L)
    qT_all = np.ascontiguousarray(
        np.concatenate([qT_base, qT_base], axis=1))  # [32, 128, L] duplicated
    kT_base = keys.transpose(0, 2, 3, 1).reshape(B * H, E, S)
    kc = kT_base.reshape(B * H, E, S // 128, 128)
    kT_all = np.ascontiguousarray(
        np.concatenate([kc[:, :, 0::2, :], kc[:, :, 1::2, :]], axis=1))
    # kT_all: [32, 128, 8, 128]; rows 0:64 = even chunks, 64:128 = odd
    v_all = np.ascontiguousarray(
        values.transpose(0, 2, 1, 3).reshape(B * H, S, E))

    nc = _get_program()
    in_maps = []
    for c in range(NCORES):
        lo = c * NP
        tau_pairs = np.ascontiguousarray(
            tau_flat[[(lo + i) // H for i in range(NP)]].reshape(1, NP))
        in_maps.append({
            "qt": qT_all[lo:lo + NP],
            "kt": kT_all[lo:lo + NP],
            "v": v_all[lo:lo + NP],
            "taus": tau_pairs,
        })

    kwargs = {}
    if os.environ.get("ATTN_TRACE"):
        kwargs["trace"] = True
        tmpdir = os.environ.get("ATTN_TRACE_DIR")
        if tmpdir:
            os.makedirs(tmpdir, exist_ok=True)
            kwargs["tmpdir"] = tmpdir

    res = run_bass_kernel_spmd(nc, in_maps, list(range(NCORES)), **kwargs)
    global LAST_RESULTS
    LAST_RESULTS = res

    o_all = np.concatenate([r["o"] for r in res.results], axis=0)  # [32, E, L]
    out = o_all.reshape(B, H, E, L).transpose(0, 3, 1, 2)  # [B, L, H, E]
    return np.ascontiguousarray(out)


# revision 6
# speedup vs baseline: 1.4032x; 1.1155x over previous
"""Multi-head attention kernel for Trainium2 (Bass/Tile), 8-core SPMD.

Problem: B=4, L=S=2048, H=8, E=D=64, fp32.
  scores = einsum('blhe,bshe->bhls', Q, K) * tau[b] + delta[b]
  A = softmax(scores / sqrt(E), axis=-1)
  out = einsum('bhls,bshd->blhd', A, V)

Key observations:
  - softmax(a*x + c) == softmax(a*x): the per-batch delta bias cancels.
  - attn_mask is all-False / unused by the reference.
  - B*H = 32 (b,h) pairs, each an independent L x S attention block.
    Shard 4 pairs per core across 8 cores; no cross-core comms.

Per-core kernel design (per (b,h) pair), all matmul operands fp16
(1 cycle/row PE streaming), fp32 PSUM:
  - Scores are computed TRANSPOSED: ST[s, l] chunks of [128, 512] so that
    the PV matmul can consume exp(ST) directly as the moving operand with
    full K=128 contraction (no P transposes).
  - QK row-packed: two K=64 matmuls run concurrently in PE row groups
    (0,0)/(64,0); host supplies K^T chunk pairs split across partition
    halves and Q^T duplicated on both halves.
  - exp is SPLIT between ScalarE (exact LUT exp, 10 of 16 s-chunks per
    l-tile) and VectorE (6 of 16) to break the ScalarE throughput wall
    (ScalarE alone = 1 elem/cyc/lane = 109us/core). The DVE side uses a
    bias-corrected Schraudolph bit-trick: one tensor_scalar (mult+add,
    fp32 in, int16 round-convert out) produces the fp16 BIT PATTERN of
    2^(y+c); c = -0.0561 zeroes the mean log-bias so the sawtooth error
    (~2% rms on 6/16 of the weights -> ~1% final L2) does not shift
    attention mass between exact and approx chunks.
  - Score PSUM: three 2-bank pools rotate over eight 2-chunk groups per
    l-tile (pool = global group index % 3, so reuse distance is always 3
    groups and no consumer sits on the next tile's critical path).
    Consumer pattern per tile: S,S,D,S,S,D,S,D. PSUM budget:
    3x2 (scores) + 2x1 (O^T) = 8 banks.
  - PV: lhsT = V' chunk [s=128, 65] where column 64 is all-ones (computes
    softmax denominators for free), rhs = exp chunk [128, 512],
    accumulated over 16 s-chunks into O^T [65, 512] PSUM.
  - Tail: copy the denominator row to SBUF, reciprocal_approx_fast (one
    custom-DVE op, ~5x faster than iterative reciprocal), GpSimd
    partition-broadcast, multiply straight out of PSUM, store O^T (host
    transposes).
  - Emission is software-pipelined (QK of unit u+1 before PV of unit u):
    PE executes in program order, so PV's wait on exp must not stall the
    next group's ready QK work.
  - Ramp: pair-0 loads are chunked (kt 2, qt 4, vp 2 pieces) so the
    first QK starts after ~1/4 of the bytes; later pairs use single
    DMAs (less GpSimd descriptor-gen) prefetched a full l-tile ahead;
    a dummy exp warms the ACT table during the initial load.

Host side only reshapes/slices/transposes (sharding + layout); all math
is on-device.
"""

import os
import numpy as np

B, L, S, H, E = 4, 2048, 2048, 8, 64
NCORES = 8
NP = (B * H) // NCORES  # pairs per core = 4

LT = 512          # l-tile size (columns of ST chunks / PV moving dim)
NLT = L // LT     # 4
NSC = S // 128    # 16 s-chunks
NG = 8            # exp groups per l-tile (2 chunks each)
# Consumers per tile: 'S' = ScalarE exact exp, 'D' = DVE Schraudolph.
CONSUMERS = "SSDSSDSD"

LOG2E = 1.4426950408889634
SCH_C = -0.0561  # bias-free Schraudolph constant (round-to-nearest)
SCH_BIAS = (15.0 + SCH_C) * 1024.0

_PROGRAM = None
LAST_RESULTS = None  # test harness reads exec_time_ns / trace path from here


def _build_program():
    import concourse.bass as bass
    import concourse.bacc as bacc
    import concourse.tile as tile
    from concourse import mybir

    f32 = mybir.dt.float32
    f16 = mybir.dt.float16
    i16 = mybir.dt.int16
    EXP = mybir.ActivationFunctionType.Exp
    MUL = mybir.AluOpType.mult
    ADD = mybir.AluOpType.add

    nc = bacc.Bacc("TRN2", target_bir_lowering=False, debug=False,
                   num_devices=NCORES)
    # qt: Q^T duplicated on both partition halves [128, L].
    # kt: K^T chunk pairs: slot j = (chunk 2j on rows 0:64, 2j+1 on 64:128).
    qT = nc.dram_tensor("qt", [NP, 2 * E, L], f32, kind="ExternalInput").ap()
    kT = nc.dram_tensor("kt", [NP, 2 * E, NSC // 2, 128], f32,
                        kind="ExternalInput").ap()
    v = nc.dram_tensor("v", [NP, S, E], f32, kind="ExternalInput").ap()
    taus = nc.dram_tensor("taus", [1, NP], f32, kind="ExternalInput").ap()
    o = nc.dram_tensor("o", [NP, E, L], f32, kind="ExternalOutput").ap()

    with tile.TileContext(nc) as tc:
        from contextlib import ExitStack
        with ExitStack() as ctx:
            consts = ctx.enter_context(tc.tile_pool(name="consts", bufs=1))
            kq_pool = ctx.enter_context(tc.tile_pool(name="kq", bufs=2))
            v_pool = ctx.enter_context(tc.tile_pool(name="vp", bufs=2))
            exp_pool = ctx.enter_context(tc.tile_pool(name="expp", bufs=3))
            tail_pool = ctx.enter_context(tc.tile_pool(name="tail", bufs=2))
            st_pools = [
                ctx.enter_context(
                    tc.tile_pool(name=f"st{chr(65 + k)}", bufs=1,
                                 space="PSUM"))
                for k in range(3)
            ]
            ot_pool = ctx.enter_context(
                tc.tile_pool(name="ot", bufs=2, space="PSUM"))

            # Warm the exp ACT table set while the first loads run.
            warm = consts.tile([128, 1], f32)
            nc.vector.memset(warm, 0.0)
            warm2 = consts.tile([128, 1], f32)
            nc.scalar.activation(warm2, warm, EXP)

            # Per-pair constants from tau[b]: ACT scale tau/sqrt(E) and the
            # Schraudolph multiplier tau/sqrt(E)*log2(e)*1024.
            tau_bc = consts.tile([128, NP], f32)
            nc.gpsimd.dma_start(out=tau_bc, in_=taus.to_broadcast([128, NP]))
            a_act = consts.tile([128, NP], f32)
            nc.scalar.mul(a_act, tau_bc, 1.0 / np.sqrt(float(E)))
            a_ts = consts.tile([128, NP], f32)
            nc.scalar.mul(a_ts, tau_bc, LOG2E * 1024.0 / np.sqrt(float(E)))

            ones_col = consts.tile([128, 1], f32)
            nc.vector.memset(ones_col, 1.0)

            units = []
            for p in range(NP):
                for t in range(NLT):
                    for gi in range(NG):
                        units.append((p, t, gi))

            pair_tiles = {}

            def emit_loads(p, chunked):
                # fp16 operands; gpsimd DMA casts f32 -> f16 in flight.
                # Pair 0 is chunked so the first QK starts early; later
                # pairs prefetch with single descriptors.
                kt_sb = kq_pool.tile([128, NSC // 2, 128], f16, tag="kt")
                qt_sb = kq_pool.tile([128, L], f16, tag="qt")
                vp_sb = v_pool.tile([128, NSC, E + 1], f16, tag="vp")
                nc.vector.tensor_copy(vp_sb[:, :, E:E + 1],
                                      ones_col.to_broadcast([128, NSC, 1]))
                vre = v[p].rearrange("(n q) e -> q n e", q=128)
                if chunked:
                    nc.gpsimd.dma_start(out=kt_sb[:, 0:1, :],
                                        in_=kT[p][:, 0:1, :])
                    nc.gpsimd.dma_start(out=kt_sb[:, 1:4, :],
                                        in_=kT[p][:, 1:4, :])
                    nc.gpsimd.dma_start(out=kt_sb[:, 4:8, :],
                                        in_=kT[p][:, 4:8, :])
                    for t in range(NLT):
                        nc.gpsimd.dma_start(
                            out=qt_sb[:, t * LT:(t + 1) * LT],
                            in_=qT[p][:, t * LT:(t + 1) * LT])
                    nc.gpsimd.dma_start(out=vp_sb[:, 0:8, 0:E],
                                        in_=vre[:, 0:8, :])
                    nc.gpsimd.dma_start(out=vp_sb[:, 8:16, 0:E],
                                        in_=vre[:, 8:16, :])
                else:
                    nc.gpsimd.dma_start(out=kt_sb, in_=kT[p])
                    nc.gpsimd.dma_start(out=qt_sb, in_=qT[p])
                    nc.gpsimd.dma_start(out=vp_sb[:, :, 0:E], in_=vre)
                pair_tiles[p] = (kt_sb, qt_sb, vp_sb)

            gctr = [0]  # global group counter -> score pool rotation

            def emit_qk(u):
                p, t, gi = u
                kt_sb, qt_sb, _ = pair_tiles[p]
                pool = st_pools[gctr[0] % 3]
                gctr[0] += 1
                st_ps = pool.tile([128, 2 * LT], f32, tag="st")
                nc.tensor.matmul(
                    st_ps[:, 0:LT],
                    lhsT=kt_sb[0:64, gi, :],
                    rhs=qt_sb[0:64, t * LT:(t + 1) * LT],
                    start=True, stop=True, tile_position=(0, 0))
                nc.tensor.matmul(
                    st_ps[:, LT:2 * LT],
                    lhsT=kt_sb[64:128, gi, :],
                    rhs=qt_sb[64:128, t * LT:(t + 1) * LT],
                    start=True, stop=True, tile_position=(64, 0))
                return st_ps

            cur_ot = [None]

            def emit_exp(u, st_ps):
                p, t, gi = u
                ex = exp_pool.tile([128, 2 * LT], f16, tag="ex")
                if CONSUMERS[gi] == "S":
                    nc.scalar.activation(ex, st_ps, EXP,
                                         scale=a_act[:, p:p + 1])
                else:
                    # Schraudolph: fp16 bits of 2^(score*a*log2e + 15+c)
                    # via fp32 affine + round-to-nearest int16 convert.
                    nc.vector.tensor_scalar(
                        ex.bitcast(i16), st_ps, a_ts[:, p:p + 1], SCH_BIAS,
                        MUL, ADD)
                return ex

            def emit_pv(u, ex):
                p, t, gi = u
                vp_sb = pair_tiles[p][2]
                if gi == 0:
                    cur_ot[0] = ot_pool.tile([E + 1, LT], f32, name="ot_ps",
                                             tag="ot_ps")
                for k in range(2):
                    c = 2 * gi + k
                    nc.tensor.matmul(
                        cur_ot[0],
                        lhsT=vp_sb[:, c, :],
                        rhs=ex[:, k * LT:(k + 1) * LT],
                        start=(c == 0), stop=(c == NSC - 1))

            def emit_tail(u):
                p, t, gi = u
                # Normalize O^T by its denominator row, store O^T (host
                # transposes back). recip row partition-broadcast on GpSimd.
                den_row = tail_pool.tile([1, LT], f32, tag="drow")
                nc.vector.tensor_copy(den_row, cur_ot[0][E:E + 1, :])
                rs_row = tail_pool.tile([1, LT], f32, tag="rsrow")
                nc.vector.reciprocal_approx_fast(rs_row, den_row)
                rb = tail_pool.tile([64, LT], f32, tag="rb")
                nc.gpsimd.partition_broadcast(rb, rs_row, channels=64)
                on = tail_pool.tile([64, LT], f32, tag="on")
                nc.vector.tensor_mul(on, cur_ot[0][0:E, :], rb)
                nc.sync.dma_start(out=o[p, :, t * LT:(t + 1) * LT], in_=on)

            # Two-unit QK lookahead: PV(u)'s FIFO wait on exp(u) must not
            # stall the next TWO groups' ready QK work (3 score pools ->
            # pool (u+2)%3 was freed by consumer(u-1), which is done by
            # the time QK(u+2) issues; lookahead 3 would deadlock on the
            # pool still being read by exp(u)).
            emit_loads(0, chunked=True)
            st_tiles = {0: emit_qk(units[0]), 1: emit_qk(units[1])}
            for i, u in enumerate(units):
                p, t, gi = u
                ex = emit_exp(u, st_tiles.pop(i))
                if i + 2 < len(units):
                    # Prefetch next pair's inputs at this pair's last l-tile.
                    if (t, gi) == (NLT - 1, 0) and p + 1 < NP:
                        emit_loads(p + 1, chunked=False)
                    st_tiles[i + 2] = emit_qk(units[i + 2])
                emit_pv(u, ex)
                if gi == NG - 1:
                    emit_tail(u)
    nc.compile()
    return nc


def _get_program():
    global _PROGRAM
    if _PROGRAM is None:
        _PROGRAM = _build_program()
    return _PROGRAM


def kernel(queries, keys, values, attn_mask=None, tau=None, delta=None):
    from concourse.bass_utils import run_bass_kernel_spmd

    queries = np.ascontiguousarray(np.asarray(queries, dtype=np.float32))
    keys = np.ascontiguousarray(np.asarray(keys, dtype=np.float32))
    values = np.ascontiguousarray(np.asarray(values, dtype=np.float32))
    tau_flat = np.asarray(tau, dtype=np.float32).reshape(B)

    # pair = b*H + h; per-pair transposed layouts (host does layout only)
    qT_base = queries.transpose(0, 2, 3, 1).reshape(B * H, E, L)
    qT_all = np.ascontiguousarray(
        np.concatenate([qT_base, qT_base], axis=1))  # [32, 128, L] duplicated
    kT_base = keys.transpose(0, 2, 3, 1).reshape(B * H, E, S)
    kc = kT_base.reshape(B * H, E, S // 128, 128)
    kT_all = np.ascontiguousarray(
        np.concatenate([kc[:, :, 0::2, :], kc[:, :, 1::2, :]], axis=1))
    # kT_all: [32, 128, 8, 128]; rows 0:64 = even chunks, 64:128 = odd
    v_all = np.ascontiguousarray(
        values.transpose(0, 2, 1, 3).reshape(B * H, S, E))

    nc = _get_program()
    in_maps = []
    for c in range(NCORES):
        lo = c * NP
        tau_pairs = np.ascontiguousarray(
            tau_flat[[(lo + i) // H for i in range(NP)]].reshape(1, NP))
        in_maps.append({
            "qt": qT_all[lo:lo + NP],
            "kt": kT_all[lo:lo + NP],
            "v": v_all[lo:lo + NP],
            "taus": tau_pairs,
        })

    kwargs = {}
    if os.environ.get("ATTN_TRACE"):
        kwargs["trace"] = True
        tmpdir = os.environ.get("ATTN_TRACE_DIR")
        if tmpdir:
            os.makedirs(tmpdir, exist_ok=True)
            kwargs["tmpdir"] = tmpdir

    res = run_bass_kernel_spmd(nc, in_maps, list(range(NCORES)), **kwargs)
    global LAST_RESULTS
    LAST_RESULTS = res

    o_all = np.concatenate([r["o"] for r in res.results], axis=0)  # [32, E, L]
    out = o_all.reshape(B, H, E, L).transpose(0, 3, 1, 2)  # [B, L, H, E]
    return np.ascontiguousarray(out)


# revision 8
# speedup vs baseline: 1.4484x; 1.0322x over previous
"""Multi-head attention kernel for Trainium2 (Bass/Tile), 8-core SPMD.

Problem: B=4, L=S=2048, H=8, E=D=64, fp32.
  scores = einsum('blhe,bshe->bhls', Q, K) * tau[b] + delta[b]
  A = softmax(scores / sqrt(E), axis=-1)
  out = einsum('bhls,bshd->blhd', A, V)

Key observations:
  - softmax(a*x + c) == softmax(a*x): the per-batch delta bias cancels.
  - attn_mask is all-False / unused by the reference.
  - B*H = 32 (b,h) pairs, each an independent L x S attention block.
    Shard 4 pairs per core across 8 cores; no cross-core comms.

Per-core kernel design (per (b,h) pair), all matmul operands fp16
(1 cycle/row PE streaming), fp32 PSUM:
  - Scores are computed TRANSPOSED: ST[s, l] chunks of [128, 512] so that
    the PV matmul can consume exp(ST) directly as the moving operand with
    full K=128 contraction (no P transposes).
  - QK row-packed: two K=64 matmuls run concurrently in PE row groups
    (0,0)/(64,0); host supplies K^T chunk pairs split across partition
    halves and Q^T duplicated on both halves.
  - exp is SPLIT between ScalarE (exact LUT exp, 10 of 16 s-chunks per
    l-tile) and VectorE (6 of 16) to break the ScalarE throughput wall
    (ScalarE alone = 1 elem/cyc/lane = 109us/core). The DVE side uses a
    bias-corrected Schraudolph bit-trick: one tensor_scalar (mult+add,
    fp32 in, int16 round-convert out) produces the fp16 BIT PATTERN of
    2^(y+c); c = -0.0561 zeroes the mean log-bias so the sawtooth error
    (~2% rms on 6/16 of the weights -> ~1% final L2) does not shift
    attention mass between exact and approx chunks.
  - Score PSUM: three 2-bank pools rotate over eight 2-chunk groups per
    l-tile (pool = global group index % 3, so reuse distance is always 3
    groups and no consumer sits on the next tile's critical path).
    Consumer pattern per tile: S,S,D,S,S,D,S,D. PSUM budget:
    3x2 (scores) + 2x1 (O^T) = 8 banks.
  - PV: lhsT = V' chunk [s=128, 65] where column 64 is all-ones (computes
    softmax denominators for free), rhs = exp chunk [128, 512],
    accumulated over 16 s-chunks into O^T [65, 512] PSUM.
  - Tail: copy the denominator row to SBUF, reciprocal_approx_fast (one
    custom-DVE op, ~5x faster than iterative reciprocal), GpSimd
    partition-broadcast, multiply straight out of PSUM, store O^T (host
    transposes).
  - Emission is software-pipelined (QK of unit u+1 before PV of unit u):
    PE executes in program order, so PV's wait on exp must not stall the
    next group's ready QK work.
  - Ramp: pair-0 loads are chunked (kt 2, qt 4, vp 2 pieces) so the
    first QK starts after ~1/4 of the bytes; later pairs use single
    DMAs (less GpSimd descriptor-gen) prefetched a full l-tile ahead;
    a dummy exp warms the ACT table during the initial load.

Host side only reshapes/slices/transposes (sharding + layout); all math
is on-device.
"""

import os
import numpy as np

B, L, S, H, E = 4, 2048, 2048, 8, 64
NCORES = 8
NP = (B * H) // NCORES  # pairs per core = 4

LT = 512          # l-tile size (columns of ST chunks / PV moving dim)
NLT = L // LT     # 4
NSC = S // 128    # 16 s-chunks
NG = 8            # exp groups per l-tile (2 chunks each)
# Consumers per tile: 'S' = ScalarE exact exp, 'D' = DVE Schraudolph.
CONSUMERS = "SSDSSDSD"

LOG2E = 1.4426950408889634
SCH_C = -0.0561  # bias-free Schraudolph constant (round-to-nearest)
SCH_BIAS = (15.0 + SCH_C) * 1024.0

_PROGRAM = None
LAST_RESULTS = None  # test harness reads exec_time_ns / trace path from here


def _build_program():
    import concourse.bass as bass
    import concourse.bacc as bacc
    import concourse.tile as tile
    from concourse import mybir

    f32 = mybir.dt.float32
    f16 = mybir.dt.float16
    i16 = mybir.dt.int16
    EXP = mybir.ActivationFunctionType.Exp
    MUL = mybir.AluOpType.mult
    ADD = mybir.AluOpType.add

    nc = bacc.Bacc("TRN2", target_bir_lowering=False, debug=False,
                   num_devices=NCORES)
    # qt: Q^T duplicated on both partition halves [128, L].
    # kt: K^T chunk pairs: slot j = (chunk 2j on rows 0:64, 2j+1 on 64:128).
    qT = nc.dram_tensor("qt", [NP, 2 * E, L], f32, kind="ExternalInput").ap()
    kT = nc.dram_tensor("kt", [NP, 2 * E, NSC // 2, 128], f32,
                        kind="ExternalInput").ap()
    v = nc.dram_tensor("v", [NP, S, E], f32, kind="ExternalInput").ap()
    taus = nc.dram_tensor("taus", [1, NP], f32, kind="ExternalInput").ap()
    o = nc.dram_tensor("o", [NP, E, L], f32, kind="ExternalOutput").ap()

    with tile.TileContext(nc) as tc:
        from contextlib import ExitStack
        with ExitStack() as ctx:
            consts = ctx.enter_context(tc.tile_pool(name="consts", bufs=1))
            kq_pool = ctx.enter_context(tc.tile_pool(name="kq", bufs=2))
            v_pool = ctx.enter_context(tc.tile_pool(name="vp", bufs=2))
            exp_pool = ctx.enter_context(tc.tile_pool(name="expp", bufs=3))
            tail_pool = ctx.enter_context(tc.tile_pool(name="tail", bufs=2))
            st_pools = [
                ctx.enter_context(
                    tc.tile_pool(name=f"st{chr(65 + k)}", bufs=1,
                                 space="PSUM"))
                for k in range(3)
            ]
            ot_pool = ctx.enter_context(
                tc.tile_pool(name="ot", bufs=2, space="PSUM"))

            # Warm the exp ACT table set while the first loads run.
            warm = consts.tile([128, 1], f32)
            nc.vector.memset(warm, 0.0)
            warm2 = consts.tile([128, 1], f32)
            nc.scalar.activation(warm2, warm, EXP)

            # Per-pair constants from tau[b]: ACT scale tau/sqrt(E) and the
            # Schraudolph multiplier tau/sqrt(E)*log2(e)*1024.
            tau_bc = consts.tile([128, NP], f32)
            nc.gpsimd.dma_start(out=tau_bc, in_=taus.to_broadcast([128, NP]))
            a_act = consts.tile([128, NP], f32)
            nc.scalar.mul(a_act, tau_bc, 1.0 / np.sqrt(float(E)))
            a_ts = consts.tile([128, NP], f32)
            nc.scalar.mul(a_ts, tau_bc, LOG2E * 1024.0 / np.sqrt(float(E)))

            ones_col = consts.tile([128, 1], f32)
            nc.vector.memset(ones_col, 1.0)

            units = []
            for p in range(NP):
                for t in range(NLT):
                    for gi in range(NG):
                        units.append((p, t, gi))

            pair_tiles = {}

            def emit_loads(p, chunked):
                # fp16 operands; gpsimd DMA casts f32 -> f16 in flight.
                # Pair 0 is chunked so the first QK starts early; later
                # pairs prefetch with single descriptors.
                kt_sb = kq_pool.tile([128, NSC // 2, 128], f16, tag="kt")
                qt_sb = kq_pool.tile([128, L], f16, tag="qt")
                vp_sb = v_pool.tile([128, NSC, E + 1], f16, tag="vp")
                nc.vector.tensor_copy(vp_sb[:, :, E:E + 1],
                                      ones_col.to_broadcast([128, NSC, 1]))
                vre = v[p].rearrange("(n q) e -> q n e", q=128)
                if chunked:
                    # First QK only needs kt slot 0 + qt l-tile 0: load
                    # those first (gpsimd descriptor-gen is serial).
                    nc.gpsimd.dma_start(out=kt_sb[:, 0:1, :],
                                        in_=kT[p][:, 0:1, :])
                    nc.gpsimd.dma_start(out=qt_sb[:, 0:LT],
                                        in_=qT[p][:, 0:LT])
                    nc.gpsimd.dma_start(out=kt_sb[:, 1:4, :],
                                        in_=kT[p][:, 1:4, :])
                    nc.gpsimd.dma_start(out=kt_sb[:, 4:8, :],
                                        in_=kT[p][:, 4:8, :])
                    nc.gpsimd.dma_start(out=vp_sb[:, 0:8, 0:E],
                                        in_=vre[:, 0:8, :])
                    for t in range(1, NLT):
                        nc.gpsimd.dma_start(
                            out=qt_sb[:, t * LT:(t + 1) * LT],
                            in_=qT[p][:, t * LT:(t + 1) * LT])
                    nc.gpsimd.dma_start(out=vp_sb[:, 8:16, 0:E],
                                        in_=vre[:, 8:16, :])
                else:
                    nc.gpsimd.dma_start(out=kt_sb, in_=kT[p])
                    nc.gpsimd.dma_start(out=qt_sb, in_=qT[p])
                    nc.gpsimd.dma_start(out=vp_sb[:, :, 0:E], in_=vre)
                pair_tiles[p] = (kt_sb, qt_sb, vp_sb)

            gctr = [0]  # global group counter -> score pool rotation

            def emit_qk(u):
                p, t, gi = u
                kt_sb, qt_sb, _ = pair_tiles[p]
                pool = st_pools[gctr[0] % 3]
                gctr[0] += 1
                st_ps = pool.tile([128, 2 * LT], f32, tag="st")
                nc.tensor.matmul(
                    st_ps[:, 0:LT],
                    lhsT=kt_sb[0:64, gi, :],
                    rhs=qt_sb[0:64, t * LT:(t + 1) * LT],
                    start=True, stop=True, tile_position=(0, 0))
                nc.tensor.matmul(
                    st_ps[:, LT:2 * LT],
                    lhsT=kt_sb[64:128, gi, :],
                    rhs=qt_sb[64:128, t * LT:(t + 1) * LT],
                    start=True, stop=True, tile_position=(64, 0))
                return st_ps

            cur_ot = [None]

            def emit_exp(u, st_ps):
                p, t, gi = u
                ex = exp_pool.tile([128, 2 * LT], f16, tag="ex")
                if CONSUMERS[gi] == "S":
                    nc.scalar.activation(ex, st_ps, EXP,
                                         scale=a_act[:, p:p + 1])
                else:
                    # Schraudolph: fp16 bits of 2^(score*a*log2e + 15+c)
                    # via fp32 affine + round-to-nearest int16 convert.
                    nc.vector.tensor_scalar(
                        ex.bitcast(i16), st_ps, a_ts[:, p:p + 1], SCH_BIAS,
                        MUL, ADD)
                return ex

            def emit_pv(u, ex):
                p, t, gi = u
                vp_sb = pair_tiles[p][2]
                if gi == 0:
                    cur_ot[0] = ot_pool.tile([E + 1, LT], f32, name="ot_ps",
                                             tag="ot_ps")
                for k in range(2):
                    c = 2 * gi + k
                    nc.tensor.matmul(
                        cur_ot[0],
                        lhsT=vp_sb[:, c, :],
                        rhs=ex[:, k * LT:(k + 1) * LT],
                        start=(c == 0), stop=(c == NSC - 1))

            def emit_tail(u):
                p, t, gi = u
                # Normalize O^T by its denominator row, store O^T (host
                # transposes back). recip row partition-broadcast on GpSimd.
                den_row = tail_pool.tile([1, LT], f32, tag="drow")
                nc.vector.tensor_copy(den_row, cur_ot[0][E:E + 1, :])
                rs_row = tail_pool.tile([1, LT], f32, tag="rsrow")
                nc.vector.reciprocal_approx_fast(rs_row, den_row)
                rb = tail_pool.tile([64, LT], f32, tag="rb")
                nc.gpsimd.partition_broadcast(rb, rs_row, channels=64)
                on = tail_pool.tile([64, LT], f32, tag="on")
                nc.vector.tensor_mul(on, cur_ot[0][0:E, :], rb)
                nc.sync.dma_start(out=o[p, :, t * LT:(t + 1) * LT], in_=on)

            # Two-unit blocks with two-unit QK lookahead. PE FIFO per
            # block: [QK(j+2), QK(j+3), PV(j), PV(j+1)] — QK(j+3) waits
            # on exp(j) (its score pool, j%3, is freed by it), the same
            # dependency PV(j) has, so it adds no stall; batching QK
            # pairs/PV pairs halves the ~100ns full-array LDWEIGHTS
            # transition tax between QK row-group MMs and PV MMs.
            emit_loads(0, chunked=True)
            st_tiles = {0: emit_qk(units[0]), 1: emit_qk(units[1])}
            for j in range(0, len(units), 2):
                u0, u1 = units[j], units[j + 1]
                ex0 = emit_exp(u0, st_tiles.pop(j))
                ex1 = emit_exp(u1, st_tiles.pop(j + 1))
                # Prefetch next pair's inputs at this pair's last l-tile.
                if (u0[1], u0[2]) == (NLT - 1, 0) and u0[0] + 1 < NP:
                    emit_loads(u0[0] + 1, chunked=False)
                if j + 2 < len(units):
                    st_tiles[j + 2] = emit_qk(units[j + 2])
                if j + 3 < len(units):
                    st_tiles[j + 3] = emit_qk(units[j + 3])
                emit_pv(u0, ex0)
                emit_pv(u1, ex1)
                if u1[2] == NG - 1:
                    emit_tail(u1)
    nc.compile()
    return nc


def _get_program():
    global _PROGRAM
    if _PROGRAM is None:
        _PROGRAM = _build_program()
    return _PROGRAM


def kernel(queries, keys, values, attn_mask=None, tau=None, delta=None):
    from concourse.bass_utils import run_bass_kernel_spmd

    queries = np.ascontiguousarray(np.asarray(queries, dtype=np.float32))
    keys = np.ascontiguousarray(np.asarray(keys, dtype=np.float32))
    values = np.ascontiguousarray(np.asarray(values, dtype=np.float32))
    tau_flat = np.asarray(tau, dtype=np.float32).reshape(B)

    # pair = b*H + h; per-pair transposed layouts (host does layout only)
    qT_base = queries.transpose(0, 2, 3, 1).reshape(B * H, E, L)
    qT_all = np.ascontiguousarray(
        np.concatenate([qT_base, qT_base], axis=1))  # [32, 128, L] duplicated
    kT_base = keys.transpose(0, 2, 3, 1).reshape(B * H, E, S)
    kc = kT_base.reshape(B * H, E, S // 128, 128)
    kT_all = np.ascontiguousarray(
        np.concatenate([kc[:, :, 0::2, :], kc[:, :, 1::2, :]], axis=1))
    # kT_all: [32, 128, 8, 128]; rows 0:64 = even chunks, 64:128 = odd
    v_all = np.ascontiguousarray(
        values.transpose(0, 2, 1, 3).reshape(B * H, S, E))

    nc = _get_program()
    in_maps = []
    for c in range(NCORES):
        lo = c * NP
        tau_pairs = np.ascontiguousarray(
            tau_flat[[(lo + i) // H for i in range(NP)]].reshape(1, NP))
        in_maps.append({
            "qt": qT_all[lo:lo + NP],
            "kt": kT_all[lo:lo + NP],
            "v": v_all[lo:lo + NP],
            "taus": tau_pairs,
        })

    kwargs = {}
    if os.environ.get("ATTN_TRACE"):
        kwargs["trace"] = True
        tmpdir = os.environ.get("ATTN_TRACE_DIR")
        if tmpdir:
            os.makedirs(tmpdir, exist_ok=True)
            kwargs["tmpdir"] = tmpdir

    res = run_bass_kernel_spmd(nc, in_maps, list(range(NCORES)), **kwargs)
    global LAST_RESULTS
    LAST_RESULTS = res

    o_all = np.concatenate([r["o"] for r in res.results], axis=0)  # [32, E, L]
    out = o_all.reshape(B, H, E, L).transpose(0, 3, 1, 2)  # [B, L, H, E]
    return np.ascontiguousarray(out)
